# revision 26
# baseline (speedup 1.0000x reference)
"""Trainium2 Bass kernel for nn_LiteTransformer (sparse_attention).

Sharding (8 cores):
  - position-attention (down): N-sharded — each core computes ALL heads'
    partial softmax sums over its 512-row m_cross shard; one f32 AllReduce
    combines them, then each core finalizes x^T for its batch.
  - self-attention blocks: core c owns batch c//2, computes all 8 heads
    (pair-redundant — cheaper than per-block collectives).
  - position-attention (up) + decoder: token-sharded — each core computes
    its 512 grid tokens x 4 batches for all heads; fully local.

Host->device transfer dominates wall clock (axon tunnel ~115MB/s), so
inputs are 4 small tensors (~1MB/core):
  - mcq   (512,1024) u8 : m_cross row-shard as round(mc*255) — same
    absolute precision as bf16 on [0,1), half the bytes, exact integer
    threshold compares. Never gathered: P1 partials and P4 only need the
    local shard.
  - blob  (18,1024) bf16: encoder-input slice + percentile thresholds.
  - sheet (224,1024) bf16: 1/8 shard of all weights; AllGathered on device.
  - blobS (128,36)  f32 : per-head exp scales (-c_h/255) + biases.
Only 3 collectives total: sheet AllGather, px AllReduce, x AllGather.
"""

import numpy as np
import ml_dtypes

import jax
# run_bass_kernel_spmd builds a fresh jit closure per call; persist the XLA
# executable so repeat calls skip the ~0.5s re-compile (NEFF is already
# disk-cached separately).
jax.config.update("jax_compilation_cache_dir", "/tmp/jax_cache_kernel")
jax.config.update("jax_persistent_cache_min_entry_size_bytes", 0)
jax.config.update("jax_persistent_cache_min_compile_time_secs", 0)

import concourse.bass as bass
import concourse.mybir as mybir
import concourse.tile as tile
from concourse import bacc
from concourse.bass import ds
from concourse.bass_utils import (run_bass_kernel_spmd as _lib_run_spmd,
                                  BassKernelResults)
from concourse.masks import make_identity

BF = mybir.dt.bfloat16
F32 = mybir.dt.float32
U8 = mybir.dt.uint8
AF = mybir.ActivationFunctionType
OP = mybir.AluOpType
NPBF = ml_dtypes.bfloat16

B, RES, N, M, H, D, KD, NB = 4, 64, 4096, 1024, 8, 256, 32, 4
BN = B * N
NCORE = 8
NS = N // NCORE          # 512 grid tokens per core
INV_SQRT_K = float(1.0 / np.sqrt(np.float32(KD)))
ALL8 = [list(range(NCORE))]

# blob layout (per-core rows, width 1024 bf16)
BLOB_ROWS = 18           # 16 enc (4 rows x 4 batches, cols 0:512) + thr
R_ENC = 0                # rows b*4+f, cols 0:512
R_TDOWN = 16             # full (1024)
R_TUP = 17               # local shard, cols 0:512
# sheet layout (global rows, width 1024 bf16)
SHEET_ROWS = 1792        # 224 per core
SH_W1, SH_W2, SH_WR = 0, 256, 512
SH_QP, SH_KP, SH_VP = 768, 1024, 1280
SH_MISC = 1536           # cols 0:256 wde1 | 256:512 wdown | 512:768 wup
# misc2 (cols 768:1024): rows +0..4 wen; wde2 halves at cols 770,771 rows +8
# blobS cols
SC_NCD, SC_NCU, SC_BEN, SC_B1, SC_BC, SC_BD1, SCOLS = 0, 8, 16, 18, 26, 34, 36

_cache = {}
_exec_cache = {}


def run_bass_kernel_spmd(nc, in_maps, core_ids, **kw):
    """Same semantics as bass_utils.run_bass_kernel_spmd for the plain SPMD
    case, but keeps the jitted executable across calls (the library builds a
    fresh closure per call, costing ~0.1s of retrace + cache-deserialize).
    Inputs are still transferred and the NEFF executed on hardware each call.
    """
    n_cores = len(core_ids)
    if kw or list(core_ids) != list(range(n_cores)) or nc.dbg_addr is not None:
        return _lib_run_spmd(nc, in_maps, core_ids=core_ids, **kw)
    ent = _exec_cache.get(id(nc))
    if ent is None:
        from jax.sharding import Mesh, PartitionSpec
        from jax.experimental.shard_map import shard_map
        from concourse.bass2jax import (_bass_exec_p, install_neuronx_cc_hook,
                                        partition_id_tensor)
        install_neuronx_cc_hook()
        pname = (nc.partition_id_tensor.name if nc.partition_id_tensor
                 else None)
        in_names, out_names, out_avals, zero_outs = [], [], [], []
        for alloc in nc.m.functions[0].allocations:
            if not isinstance(alloc, mybir.MemoryLocationSet):
                continue
            name = alloc.memorylocations[0].name
            if alloc.kind == "ExternalInput":
                if name != pname:
                    in_names.append(name)
            elif alloc.kind == "ExternalOutput":
                out_names.append(name)
                shape = tuple(alloc.tensor_shape)
                dtype = mybir.dt.np(alloc.dtype)
                out_avals.append(jax.core.ShapedArray(shape, dtype))
                zero_outs.append(np.zeros(shape, dtype))
        n_params = len(in_names)
        all_names = in_names + out_names + ([pname] if pname else [])

        def _body(*args):
            operands = list(args)
            if pname is not None:
                operands.append(partition_id_tensor())
            outs = _bass_exec_p.bind(
                *operands, out_avals=tuple(out_avals),
                in_names=tuple(all_names), out_names=tuple(out_names),
                lowering_input_output_aliases=(), sim_require_finite=True,
                sim_require_nnan=True, nc=nc)
            return tuple(outs)

        devices = jax.devices()[:n_cores]
        mesh = Mesh(np.asarray(devices), ("core",))
        n_io = n_params + len(out_names)
        sharded = jax.jit(
            shard_map(_body, mesh=mesh,
                      in_specs=(PartitionSpec("core"),) * n_io,
                      out_specs=(PartitionSpec("core"),) * len(out_names),
                      check_rep=False),
            donate_argnums=tuple(range(n_params, n_io)), keep_unused=True)
        ent = (sharded, in_names, n_params, out_names, out_avals, zero_outs)
        _exec_cache[id(nc)] = ent
    sharded, in_names, n_params, out_names, out_avals, zero_outs = ent
    concat_in = [
        np.concatenate([np.asarray(in_maps[c][nm]) for c in range(n_cores)],
                       axis=0) for nm in in_names]
    concat_zeros = [np.zeros((n_cores * z.shape[0], *z.shape[1:]), z.dtype)
                    for z in zero_outs]
    out_arrs = sharded(*concat_in, *concat_zeros)
    results = [
        {nm: np.asarray(out_arrs[i]).reshape(n_cores, *out_avals[i].shape)[c]
         for i, nm in enumerate(out_names)}
        for c in range(n_cores)]
    return BassKernelResults(results=results, instructions_and_trace=None,
                             profile_json=None, exec_time_ns=None)


def _build():
    nc = bacc.Bacc("TRN2", target_bir_lowering=False, debug=False,
                   num_devices=NCORE)

    mcq = nc.dram_tensor("mcq", [NS, 1024], U8, kind="ExternalInput").ap()
    blob = nc.dram_tensor("blob", [BLOB_ROWS, 1024], BF,
                          kind="ExternalInput").ap()
    sheet = nc.dram_tensor("sheet", [SHEET_ROWS // NCORE, 1024], BF,
                           kind="ExternalInput").ap()
    blobS = nc.dram_tensor("blobS", [128, SCOLS], F32,
                           kind="ExternalInput").ap()
    out_shard = nc.dram_tensor("out_shard", [1, BN // NCORE], F32,
                               kind="ExternalOutput").ap()

    with tile.TileContext(nc) as tc:
        with (
            tc.tile_pool(name="dram", bufs=1, space="DRAM") as dram,
            tc.tile_pool(name="consts", bufs=1) as consts,
            tc.tile_pool(name="small", bufs=6) as small,
            tc.tile_pool(name="pp", bufs=4, space="PSUM") as pp,
            tc.tile_pool(name="pt", bufs=2, space="PSUM") as ppt,
        ):
            ident = consts.tile([128, 128], BF, name="ident", tag="ident")
            make_identity(nc, ident)
            pid = nc.sync.partition_id()

            # ---- gather the weight sheet across cores ----
            sheet_in = dram.tile([SHEET_ROWS // NCORE, 1024], BF,
                                 name="sheeti", tag="sheeti")
            nc.sync.dma_start(sheet_in[:, :], sheet[:, :])
            sheet_out = dram.tile([SHEET_ROWS, 1024], BF, name="sheeto",
                                  tag="sheeto", addr_space="Shared")
            nc.gpsimd.collective_compute(
                "AllGather", OP.bypass, replica_groups=ALL8,
                ins=[sheet_in.opt()], outs=[sheet_out.opt()])

            blobS_sb = consts.tile([128, SCOLS], F32, name="bS", tag="bS")
            nc.sync.dma_start(blobS_sb[:], blobS[:, :])

            # threshold rows broadcast to 128 partitions (ones ⊗ row matmul)
            ones_sb = consts.tile([1, 128], BF, name="ones", tag="ones")
            nc.vector.memset(ones_sb[:], 1.0)
            td_row = consts.tile([1, 1024], BF, name="tdr", tag="tdr")
            nc.sync.dma_start(td_row[:], blob[R_TDOWN:R_TDOWN + 1, :])
            thrD = consts.tile([128, 1024], BF, name="thrD", tag="thrD")
            for hf in range(2):
                pb = pp.tile([128, 512], F32, name="pp", tag="pp")
                nc.tensor.matmul(pb[:], ones_sb[:],
                                 td_row[:, hf * 512:(hf + 1) * 512])
                nc.vector.tensor_copy(thrD[:, hf * 512:(hf + 1) * 512], pb[:])
            tu_row = consts.tile([1, NS], BF, name="tur", tag="tur")
            nc.sync.dma_start(tu_row[:], blob[R_TUP:R_TUP + 1, 0:NS])
            thrU = consts.tile([128, NS], BF, name="thrU", tag="thrU")
            pb = pp.tile([128, 512], F32, name="pp", tag="pp")
            nc.tensor.matmul(pb[:], ones_sb[:], tu_row[:])
            nc.vector.tensor_copy(thrU[:], pb[:])

            wen_sb = consts.tile([4, 256], BF, name="wen", tag="wen")
            nc.sync.dma_start(wen_sb[:],
                              sheet_out[SH_MISC:SH_MISC + 4, 768:1024])
            wdna, wupa = [], []
            for t in range(2):
                w = consts.tile([128, 256], BF, name=f"wdn{t}", tag=f"wdn{t}")
                nc.sync.dma_start(
                    w[:], sheet_out[SH_MISC + t * 128:SH_MISC + (t + 1) * 128,
                                    256:512])
                wdna.append(w)
                w = consts.tile([128, 256], BF, name=f"wupt{t}",
                                tag=f"wupt{t}")
                nc.sync.dma_start(
                    w[:], sheet_out[SH_MISC + t * 128:SH_MISC + (t + 1) * 128,
                                    512:768])
                wupa.append(w)
            ben_sb = [blobS_sb[:, SC_BEN + t:SC_BEN + t + 1] for t in range(2)]

            # local bf16 m_cross scratch (for P4's transposed reads)
            mc_loc = dram.tile([NS, 1024], BF, name="mcd", tag="mcd")
            ar_in = dram.tile([128, H * 8 * 132], F32, name="ari", tag="ari")
            ar_out = dram.tile([128, H * 8 * 132], F32, name="aro", tag="aro",
                               addr_space="Shared")
            ag3_in = dram.tile([D, M], BF, name="ag3i", tag="ag3i")
            ag3_out = dram.tile([NCORE * D, M], BF, name="ag3o", tag="ag3o",
                                addr_space="Shared")

            def psum(p, f, dt=F32):
                return pp.tile([p, f], dt, name="pp", tag="pp")

            _lwn = [0]

            def lw(pool, p0, p1, f0, f1, dt=BF):
                _lwn[0] += 1
                t = pool.tile([p1 - p0, f1 - f0], dt, name=f"lw{_lwn[0]}",
                              tag=f"lw{_lwn[0]}")
                nc.sync.dma_start(t[:], sheet_out[p0:p1, f0:f1])
                return t

            # ------- P1: down partial sums over local N-shard, all heads ----
            with tc.tile_pool(name="p1", bufs=3) as p1, \
                 tc.tile_pool(name="p1s", bufs=2) as p1s, \
                 tc.tile_pool(name="p1keep", bufs=1) as p1k:
                enc_loc = p1k.tile([4, 4 * NS], BF, name="encl", tag="encl")
                for b in range(B):
                    nc.sync.dma_start(enc_loc[:, b * NS:(b + 1) * NS],
                                      blob[R_ENC + b * 4:R_ENC + b * 4 + 4,
                                           0:NS])

                # v_all[nl]: (128, 1056) cols h*132 + b*33 + k (col 32 = ones)
                v_all = [p1k.tile([128, H * 132], BF, name=f"va{i}",
                                  tag=f"va{i}") for i in range(4)]
                for b in range(B):
                    for nl in range(4):
                        off = b * NS + nl * 128
                        enT = []
                        for t in range(2):
                            pe = psum(128, 128)
                            nc.tensor.matmul(
                                pe[:], wen_sb[:, t * 128:(t + 1) * 128],
                                enc_loc[:, off:off + 128])
                            g = p1.tile([128, 128], BF, name="enT", tag="enT")
                            nc.scalar.activation(g[:], pe[:], AF.Gelu,
                                                 bias=ben_sb[t])
                            enT.append(g)
                        pv = psum(128, 256)
                        for t in range(2):
                            nc.tensor.matmul(pv[:], enT[t][:], wdna[t][:],
                                             start=(t == 0), stop=(t == 1))
                        for h in range(H):
                            nc.vector.tensor_copy(
                                v_all[nl][:, h * 132 + b * 33:
                                          h * 132 + b * 33 + KD],
                                pv[:, h * 32:(h + 1) * 32])
                        if b == 0:
                            for h in range(H):
                                for bb in range(B):
                                    nc.vector.memset(
                                        v_all[nl][:, h * 132 + bb * 33 + 32:
                                                  h * 132 + bb * 33 + 33],
                                        1.0)

                # local mc tiles: u8 -> bf16 (+ write-through for P4), masks
                mct, msk = [], []
                for nl in range(4):
                    mq = p1.tile([128, M], U8, name="mq", tag="mq")
                    nc.sync.dma_start(mq[:], mcq[nl * 128:(nl + 1) * 128, :])
                    t = p1k.tile([128, M], BF, name=f"mct{nl}", tag=f"mct{nl}")
                    nc.vector.tensor_copy(t[:], mq[:])
                    nc.sync.dma_start(mc_loc[nl * 128:(nl + 1) * 128, :], t[:])
                    m = p1k.tile([128, M], BF, name=f"msk{nl}", tag=f"msk{nl}")
                    nc.vector.tensor_tensor(m[:], t[:], thrD[:], OP.is_le)
                    mct.append(t)
                    msk.append(m)

                # per head: a = exp(-c_h/255 q) * mask; px partials -> pxs
                pxs = p1k.tile([128, H * 8 * 132], F32, name="pxs", tag="pxs")
                for h in range(H):
                    ah = [p1s.tile([128, M], BF, name=f"ah{nl}", tag=f"ah{nl}")
                          for nl in range(4)]
                    for nl in range(4):
                        nc.scalar.activation(
                            ah[nl][:], mct[nl][:], AF.Exp,
                            scale=blobS_sb[:, SC_NCD + h:SC_NCD + h + 1])
                        nc.vector.tensor_tensor(ah[nl][:], ah[nl][:],
                                                msk[nl][:], OP.mult)
                    for mi in range(8):
                        pa = psum(128, 132)
                        for nl in range(4):
                            nc.tensor.matmul(
                                pa[:], ah[nl][:, mi * 128:(mi + 1) * 128],
                                v_all[nl][:, h * 132:(h + 1) * 132],
                                start=(nl == 0), stop=(nl == 3))
                        nc.vector.tensor_copy(
                            pxs[:, (h * 8 + mi) * 132:(h * 8 + mi + 1) * 132],
                            pa[:])
                nc.sync.dma_start(ar_in[:], pxs[:])
                nc.gpsimd.collective_compute(
                    "AllReduce", OP.add, replica_groups=ALL8,
                    ins=[ar_in.opt()], outs=[ar_out.opt()])

            # ---------------- P2: blocks ----------------
            # every core computes ALL 8 heads for its batch (pid//2)
            b0c = (pid // 2) * 33
            with tc.tile_pool(name="p2", bufs=2) as p2, \
                 tc.tile_pool(name="p2s", bufs=2) as p2s, \
                 tc.tile_pool(name="p2e", bufs=1) as p2e:
                # finalize x^T for my batch from the reduced sums
                xT = [p2e.tile([128, M], BF, name=f"xT{t}", tag=f"xT{t}")
                      for t in range(2)]
                for h in range(H):
                    for mi in range(8):
                        nb = small.tile([128, 33], F32, name="nb", tag="nb")
                        nc.sync.dma_start(
                            nb[:], ar_out[0:128,
                                          ds((h * 8 + mi) * 132 + b0c, 33)])
                        rc = small.tile([128, 1], F32, name="rc", tag="rc")
                        nc.vector.reciprocal(rc[:], nb[:, 32:33])
                        gx = small.tile([128, KD], BF, name="gx", tag="gx")
                        nc.scalar.activation(gx[:], nb[:, 0:KD], AF.Gelu,
                                             scale=rc[:])
                        ptr = ppt.tile([KD, 128], BF, name="tp", tag="tp")
                        nc.tensor.transpose(ptr[:], gx[:], ident[:])
                        nc.vector.tensor_copy(
                            xT[h // 4][(h % 4) * 32:(h % 4) * 32 + 32,
                                       mi * 128:(mi + 1) * 128], ptr[:])

                for blk in range(NB):
                    qp_sb, kp_sb, vp_sb = [], [], []
                    for t in range(2):
                        for dst, base in ((qp_sb, SH_QP), (kp_sb, SH_KP),
                                          (vp_sb, SH_VP)):
                            _lwn[0] += 1
                            w = p2.tile([128, 256], BF, name=f"lw{_lwn[0]}",
                                        tag=f"lw{_lwn[0]}")
                            nc.sync.dma_start(
                                w[:], sheet_out[base + t * 128:
                                                base + (t + 1) * 128,
                                                blk * 256:(blk + 1) * 256])
                            dst.append(w)

                    qt = [p2e.tile([128, M], BF, name=f"qt{g}", tag=f"qt{g}")
                          for g in range(2)]
                    kt = [p2e.tile([128, M], BF, name=f"kt{g}", tag=f"kt{g}")
                          for g in range(2)]
                    for dst2, wsb in ((qt, qp_sb), (kt, kp_sb)):
                        for g in range(2):
                            for mh in range(2):
                                pq = psum(128, 512)
                                for t in range(2):
                                    nc.tensor.matmul(
                                        pq[:],
                                        wsb[t][:, g * 128:(g + 1) * 128],
                                        xT[t][:, mh * 512:(mh + 1) * 512],
                                        start=(t == 0), stop=(t == 1))
                                nc.vector.tensor_copy(
                                    dst2[g][:, mh * 512:(mh + 1) * 512],
                                    pq[:])
                    qh = [p2e.tile([KD, M], BF, name=f"qh{h}", tag=f"qh{h}")
                          for h in range(H)]
                    kh = [p2e.tile([KD, M], BF, name=f"kh{h}", tag=f"kh{h}")
                          for h in range(H)]
                    for h in range(H):
                        nc.vector.tensor_copy(
                            qh[h][:], qt[h // 4][(h % 4) * 32:
                                                 (h % 4) * 32 + 32, :])
                        nc.vector.tensor_copy(
                            kh[h][:], kt[h // 4][(h % 4) * 32:
                                                 (h % 4) * 32 + 32, :])

                    vh = [p2e.tile([128, H * 33], BF, name=f"vh{ni}",
                                   tag=f"vh{ni}") for ni in range(8)]
                    for ni in range(8):
                        pvv = psum(128, 256)
                        for t in range(2):
                            nc.tensor.matmul(
                                pvv[:],
                                xT[t][:, ni * 128:(ni + 1) * 128],
                                vp_sb[t][:], start=(t == 0), stop=(t == 1))
                        for h in range(H):
                            nc.vector.tensor_copy(
                                vh[ni][:, h * 33:h * 33 + KD],
                                pvv[:, h * 32:h * 32 + 32])
                            nc.vector.memset(
                                vh[ni][:, h * 33 + 32:h * 33 + 33], 1.0)

                    paT = [p2e.tile([128, M], BF, name=f"paT{t}",
                                    tag=f"paT{t}") for t in range(2)]
                    for h in range(H):
                        es = [p2s.tile([128, M], BF, name=f"es{ni}",
                                       tag=f"es{ni}") for ni in range(8)]
                        for ni in range(8):
                            for mh in range(2):
                                psc = psum(128, 512)
                                nc.tensor.matmul(
                                    psc[:],
                                    kh[h][:, ni * 128:(ni + 1) * 128],
                                    qh[h][:, mh * 512:(mh + 1) * 512])
                                nc.scalar.activation(
                                    es[ni][:, mh * 512:(mh + 1) * 512],
                                    psc[:], AF.Exp, scale=INV_SQRT_K)
                        for mi in range(8):
                            pa = psum(128, 33)
                            for ni in range(8):
                                nc.tensor.matmul(
                                    pa[:],
                                    es[ni][:, mi * 128:(mi + 1) * 128],
                                    vh[ni][:, h * 33:h * 33 + 33],
                                    start=(ni == 0), stop=(ni == 7))
                            rc = small.tile([128, 1], F32, name="rc", tag="rc")
                            nc.vector.reciprocal(rc[:], pa[:, 32:33])
                            gx = small.tile([128, KD], BF, name="gx", tag="gx")
                            nc.scalar.activation(gx[:], pa[:, 0:KD], AF.Gelu,
                                                 scale=rc[:])
                            ptr = ppt.tile([KD, 128], BF, name="tp", tag="tp")
                            nc.tensor.transpose(ptr[:], gx[:], ident[:])
                            nc.vector.tensor_copy(
                                paT[h // 4][(h % 4) * 32:(h % 4) * 32 + 32,
                                            mi * 128:(mi + 1) * 128], ptr[:])

                    w1_sb = [[lw(p2, SH_W1 + i * 128, SH_W1 + (i + 1) * 128,
                                 blk * 256 + o * 128, blk * 256 + (o + 1) * 128)
                              for o in range(2)] for i in range(2)]
                    b1_sb = [blobS_sb[:, SC_B1 + blk * 2 + t:
                                      SC_B1 + blk * 2 + t + 1]
                             for t in range(2)]
                    h1 = [p2e.tile([128, M], BF, name=f"h1{t}", tag=f"h1{t}")
                          for t in range(2)]
                    for o in range(2):
                        for mh in range(2):
                            ph = psum(128, 512)
                            for i in range(2):
                                nc.tensor.matmul(
                                    ph[:], w1_sb[i][o][:],
                                    paT[i][:, mh * 512:(mh + 1) * 512],
                                    start=(i == 0), stop=(i == 1))
                            nc.scalar.activation(
                                h1[o][:, mh * 512:(mh + 1) * 512], ph[:],
                                AF.Gelu, bias=b1_sb[o])

                    w2_sb = [[lw(p2, SH_W2 + i * 128, SH_W2 + (i + 1) * 128,
                                 blk * 256 + o * 128, blk * 256 + (o + 1) * 128)
                              for o in range(2)] for i in range(2)]
                    wr_sb = [[lw(p2, SH_WR + i * 128, SH_WR + (i + 1) * 128,
                                 blk * 256 + o * 128, blk * 256 + (o + 1) * 128)
                              for o in range(2)] for i in range(2)]
                    bc_sb = [blobS_sb[:, SC_BC + blk * 2 + t:
                                      SC_BC + blk * 2 + t + 1]
                             for t in range(2)]
                    xn = [p2e.tile([128, M], BF, name=f"xn{t}", tag=f"xn{t}")
                          for t in range(2)]
                    for o in range(2):
                        for mh in range(2):
                            po = psum(128, 512)
                            nc.tensor.matmul(
                                po[:], w2_sb[0][o][:],
                                h1[0][:, mh * 512:(mh + 1) * 512],
                                start=True, stop=False)
                            nc.tensor.matmul(
                                po[:], w2_sb[1][o][:],
                                h1[1][:, mh * 512:(mh + 1) * 512],
                                start=False, stop=False)
                            nc.tensor.matmul(
                                po[:], wr_sb[0][o][:],
                                xT[0][:, mh * 512:(mh + 1) * 512],
                                start=False, stop=False)
                            nc.tensor.matmul(
                                po[:], wr_sb[1][o][:],
                                xT[1][:, mh * 512:(mh + 1) * 512],
                                start=False, stop=True)
                            nc.scalar.activation(
                                xn[o][:, mh * 512:(mh + 1) * 512], po[:],
                                AF.Gelu, bias=bc_sb[o])
                    xT = xn

                for t in range(2):
                    nc.sync.dma_start(ag3_in[t * 128:(t + 1) * 128, :],
                                      xT[t][:])
                nc.gpsimd.collective_compute(
                    "AllGather", OP.bypass, replica_groups=ALL8,
                    ins=[ag3_in.opt()], outs=[ag3_out.opt()])

            # ------- P4+P5: up on local token shard (all heads) + decoder ---
            TS = BN // NCORE  # 2048 = 4 batches x 512 local tokens
            with tc.tile_pool(name="p4", bufs=3) as p4, \
                 tc.tile_pool(name="p4s", bufs=2) as p4s, \
                 tc.tile_pool(name="p4keep", bufs=1) as p4k:
                # vu_all[mi]: (128, 1056) cols h*132 + b*33 + k
                vu = [p4k.tile([128, H * 132], BF, name=f"vu{i}",
                               tag=f"vu{i}") for i in range(8)]
                for b in range(B):
                    xb = [p4.tile([128, M], BF, name=f"xb{t}", tag=f"xb{t}")
                          for t in range(2)]
                    for t in range(2):
                        nc.sync.dma_start(
                            xb[t][:],
                            ag3_out[2 * b * D + t * 128:
                                    2 * b * D + (t + 1) * 128, :])
                    for mi in range(8):
                        pv = psum(128, 256)
                        for t in range(2):
                            nc.tensor.matmul(
                                pv[:], xb[t][:, mi * 128:(mi + 1) * 128],
                                wupa[t][:], start=(t == 0), stop=(t == 1))
                        for h in range(H):
                            nc.vector.tensor_copy(
                                vu[mi][:, h * 132 + b * 33:
                                       h * 132 + b * 33 + KD],
                                pv[:, h * 32:(h + 1) * 32])
                        if b == 0:
                            for h in range(H):
                                for bb in range(B):
                                    nc.vector.memset(
                                        vu[mi][:, h * 132 + bb * 33 + 32:
                                               h * 132 + bb * 33 + 33], 1.0)

                # local mc^T tiles (128 M-part, 512 local N) + masks
                mctT, mskU = [], []
                for ki in range(8):
                    t = p4k.tile([128, NS], BF, name=f"mT{ki}", tag=f"mT{ki}")
                    nc.sync.dma_start(t[:],
                                      mc_loc[:, ki * 128:(ki + 1) * 128],
                                      transpose=True)
                    m = p4k.tile([128, NS], BF, name=f"mU{ki}", tag=f"mU{ki}")
                    nc.vector.tensor_tensor(m[:], t[:], thrU[:], OP.is_le)
                    mctT.append(t)
                    mskU.append(m)

                deT = [p4k.tile([128, TS], BF, name=f"deT{t}", tag=f"deT{t}")
                       for t in range(2)]
                for h in range(H):
                    eh = [p4s.tile([128, NS], BF, name=f"eh{ki}",
                                   tag=f"eh{ki}") for ki in range(8)]
                    for ki in range(8):
                        nc.scalar.activation(
                            eh[ki][:], mctT[ki][:], AF.Exp,
                            scale=blobS_sb[:, SC_NCU + h:SC_NCU + h + 1])
                        nc.vector.tensor_tensor(eh[ki][:], eh[ki][:],
                                                mskU[ki][:], OP.mult)
                    for qi in range(4):
                        pd = psum(128, 132)
                        for ki in range(8):
                            nc.tensor.matmul(
                                pd[:], eh[ki][:, qi * 128:(qi + 1) * 128],
                                vu[ki][:, h * 132:(h + 1) * 132],
                                start=(ki == 0), stop=(ki == 7))
                        for b in range(B):
                            rc = small.tile([128, 1], F32, name="rc", tag="rc")
                            nc.vector.reciprocal(
                                rc[:], pd[:, b * 33 + 32:b * 33 + 33])
                            gx = small.tile([128, KD], BF, name="gx", tag="gx")
                            nc.scalar.activation(
                                gx[:], pd[:, b * 33:b * 33 + KD],
                                AF.Gelu, scale=rc[:])
                            ptr = ppt.tile([KD, 128], BF, name="tp", tag="tp")
                            nc.tensor.transpose(ptr[:], gx[:], ident[:])
                            nc.vector.tensor_copy(
                                deT[h // 4][(h % 4) * 32:(h % 4) * 32 + 32,
                                            b * NS + qi * 128:
                                            b * NS + (qi + 1) * 128], ptr[:])

                # decoder directly on the local de^T tiles
                wd1 = [[lw(p4, SH_MISC + i * 128, SH_MISC + (i + 1) * 128,
                           o * 128, (o + 1) * 128) for o in range(2)]
                       for i in range(2)]
                bd1 = [blobS_sb[:, SC_BD1 + t:SC_BD1 + t + 1]
                       for t in range(2)]
                wd2 = [lw(p4, SH_MISC + 8, SH_MISC + 8 + 128,
                          770 + t, 771 + t) for t in range(2)]
                g = [p4k.tile([128, TS], BF, name=f"g{t}", tag=f"g{t}")
                     for t in range(2)]
                for o in range(2):
                    for th in range(4):
                        pg = psum(128, 512)
                        for i in range(2):
                            nc.tensor.matmul(
                                pg[:], wd1[i][o][:],
                                deT[i][:, th * 512:(th + 1) * 512],
                                start=(i == 0), stop=(i == 1))
                        nc.scalar.activation(
                            g[o][:, th * 512:(th + 1) * 512], pg[:],
                            AF.Gelu, bias=bd1[o])
                osb = p4k.tile([1, TS], F32, name="osb", tag="osb")
                for th in range(4):
                    p2o = psum(1, 512)
                    for i in range(2):
                        nc.tensor.matmul(
                            p2o[:], wd2[i][:],
                            g[i][:, th * 512:(th + 1) * 512],
                            start=(i == 0), stop=(i == 1))
                    nc.vector.tensor_copy(
                        osb[:, th * 512:(th + 1) * 512], p2o[:])
                nc.sync.dma_start(out_shard[:, :], osb[:])

    nc.compile()
    return nc


def _prep_inputs(inputs, m_cross, W_en, b_en, r_down, w_down, q_pa, k_pa,
                 v_pa, W1_mlp, b1_mlp, W2_mlp, b2_mlp, W_res, b_res, r_up,
                 w_up, W_de1, b_de1, W_de2, b_de2, y_mean, y_std):
    f32 = np.float32
    mc = np.asarray(m_cross, f32)
    # uint8 quantization: q = round(mc*255); same absolute precision as bf16
    # on [0,1) and exact integer threshold comparisons on device
    mcq8 = np.clip(np.round(mc * 255.0), 0, 255).astype(np.uint8)
    mcqf = mcq8.astype(f32)

    # quantization-robust percentile thresholds: max passing q value so the
    # device-side (q <= thr) comparison reproduces the f32 mask
    kd_ = int(0.30 * (N - 1))          # 1228
    vkd = np.partition(mc, kd_, axis=0)[kd_, :]               # (M,)
    t_down_dev = np.where(mc <= vkd[None, :], mcqf, -np.inf).max(axis=0)
    ku_ = int(0.30 * (M - 1))          # 306
    vku = np.partition(mc, ku_, axis=1)[:, ku_]               # (N,)
    t_up_dev = np.where(mc <= vku[:, None], mcqf, -np.inf).max(axis=1)

    # encoder input rows (4 feats x BN), sliced per core below
    gx = np.linspace(0.0, 1.0, RES + 1, dtype=f32)[:-1]
    gxx = np.broadcast_to(gx[:, None], (RES, RES))
    gyy = np.broadcast_to(gx[None, :], (RES, RES))
    encf = np.zeros((4, B, N), f32)
    encf[0] = gxx.reshape(-1)[None, :]
    encf[1] = gyy.reshape(-1)[None, :]
    encf[2] = np.asarray(inputs, f32).reshape(B, N)

    c_down = np.tan(0.25 * np.pi * (1.0 + np.sin(np.asarray(r_down, f32)
                                                 .reshape(H)))).astype(f32)
    c_up = np.tan(0.25 * np.pi * (1.0 + np.sin(np.asarray(r_up, f32)
                                               .reshape(H)))).astype(f32)

    # ---- weight sheet (1792, 1024) ----
    sheet = np.zeros((SHEET_ROWS, 1024), NPBF)

    def cat_blocks(w):  # (NB, D, D) -> (D, NB*D)
        return np.asarray(w, f32).transpose(1, 0, 2).reshape(D, NB * D)

    def cat_heads(w):   # (NB, H, D, K) -> (D, NB*H*K)
        return np.asarray(w, f32).transpose(2, 0, 1, 3).reshape(D, NB * H * KD)

    sheet[SH_W1:SH_W1 + D] = cat_blocks(W1_mlp).astype(NPBF)
    sheet[SH_W2:SH_W2 + D] = cat_blocks(W2_mlp).astype(NPBF)
    sheet[SH_WR:SH_WR + D] = cat_blocks(W_res).astype(NPBF)
    sheet[SH_QP:SH_QP + D] = cat_heads(q_pa).astype(NPBF)
    sheet[SH_KP:SH_KP + D] = cat_heads(k_pa).astype(NPBF)
    sheet[SH_VP:SH_VP + D] = cat_heads(v_pa).astype(NPBF)
    sheet[SH_MISC:SH_MISC + D, 0:256] = np.asarray(W_de1, f32).astype(NPBF)
    sheet[SH_MISC:SH_MISC + D, 256:512] = (
        np.asarray(w_down, f32).transpose(1, 0, 2).reshape(D, H * KD)
        .astype(NPBF))
    sheet[SH_MISC:SH_MISC + D, 512:768] = (
        np.asarray(w_up, f32).transpose(1, 0, 2).reshape(D, H * KD)
        .astype(NPBF))
    wen4 = np.zeros((4, D), f32)
    wen4[:3, :] = np.asarray(W_en, f32)
    sheet[SH_MISC:SH_MISC + 4, 768:1024] = wen4.astype(NPBF)
    ystd = float(np.asarray(y_std, f32))
    ymean = float(np.asarray(y_mean, f32))
    wde2f = (np.asarray(W_de2, f32).reshape(D) * ystd).astype(NPBF)
    sheet[SH_MISC + 8:SH_MISC + 8 + 128, 770] = wde2f[0:128]
    sheet[SH_MISC + 8:SH_MISC + 8 + 128, 771] = wde2f[128:256]
    bde2f = float(np.asarray(b_de2, f32).reshape(-1)[0] * ystd + ymean)

    # ---- f32 constants (128, 36) ----  (exp scales absorb the /255)
    bS = np.zeros((128, SCOLS), f32)
    bS[:, SC_NCD:SC_NCD + 8] = -c_down[None, :] / 255.0
    bS[:, SC_NCU:SC_NCU + 8] = -c_up[None, :] / 255.0
    ben = np.asarray(b_en, f32).reshape(D)
    bS[:, SC_BEN] = ben[0:128]
    bS[:, SC_BEN + 1] = ben[128:256]
    b1f = np.asarray(b1_mlp, f32).reshape(NB, D)
    bcf = (np.asarray(b2_mlp, f32) + np.asarray(b_res, f32)).reshape(NB, D)
    for blk in range(NB):
        for t in range(2):
            bS[:, SC_B1 + blk * 2 + t] = b1f[blk, t * 128:(t + 1) * 128]
            bS[:, SC_BC + blk * 2 + t] = bcf[blk, t * 128:(t + 1) * 128]
    bd1 = np.asarray(b_de1, f32).reshape(D)
    bS[:, SC_BD1] = bd1[0:128]
    bS[:, SC_BD1 + 1] = bd1[128:256]

    in_maps = []
    for c in range(NCORE):
        blob = np.zeros((BLOB_ROWS, 1024), NPBF)
        for b in range(B):
            blob[R_ENC + b * 4:R_ENC + b * 4 + 4, 0:NS] = (
                encf[:, b, c * NS:(c + 1) * NS].astype(NPBF))
        blob[R_TDOWN] = t_down_dev.astype(NPBF)
        blob[R_TUP, 0:NS] = t_up_dev[c * NS:(c + 1) * NS].astype(NPBF)
        in_maps.append({
            "mcq": mcq8[c * NS:(c + 1) * NS],
            "blob": blob,
            "sheet": sheet[c * (SHEET_ROWS // NCORE):
                           (c + 1) * (SHEET_ROWS // NCORE)],
            "blobS": bS,
        })
    return in_maps, bde2f


def kernel(**inputs):
    if "nc" not in _cache:
        _cache["nc"] = _build()
    nc = _cache["nc"]
    in_maps, bde2f = _prep_inputs(**inputs)
    res = run_bass_kernel_spmd(nc, in_maps, core_ids=list(range(NCORE)))
    out = np.empty((B, N), np.float32)
    for c in range(NCORE):
        sh = res.results[c]["out_shard"].reshape(B, NS) + np.float32(bde2f)
        out[:, c * NS:(c + 1) * NS] = sh
    return out.reshape(B, RES, RES, 1).astype(np.float32)


# revision 33
# speedup vs baseline: 1.2362x; 1.2362x over previous
"""Trainium2 Bass kernel for nn_LiteTransformer (sparse_attention).

Sharding (8 cores):
  - position-attention (down): N-sharded — each core computes ALL heads'
    partial softmax sums over its 512-row m_cross shard; one f32 AllReduce
    combines them, then each core finalizes x^T for its batch.
  - self-attention blocks: core c owns batch c//2, computes all 8 heads
    (pair-redundant — cheaper than per-block collectives).
  - position-attention (up) + decoder: token-sharded — each core computes
    its 512 grid tokens x 4 batches for all heads; fully local.

Host->device transfer dominates wall clock (axon tunnel ~115MB/s), so
inputs are 4 small tensors (~1MB/core):
  - mcq   (512,1024) u8 : m_cross row-shard as round(mc*255) — same
    absolute precision as bf16 on [0,1), half the bytes, exact integer
    threshold compares. Never gathered: P1 partials and P4 only need the
    local shard.
  - blob  (18,1024) bf16: encoder-input slice + percentile thresholds.
  - sheet (224,1024) bf16: 1/8 shard of all weights; AllGathered on device.
  - blobS (128,36)  f32 : per-head exp scales (-c_h/255) + biases.
Only 3 collectives total: sheet AllGather, px AllReduce, x AllGather.
"""

import numpy as np
import ml_dtypes

import jax
# run_bass_kernel_spmd builds a fresh jit closure per call; persist the XLA
# executable so repeat calls skip the ~0.5s re-compile (NEFF is already
# disk-cached separately).
jax.config.update("jax_compilation_cache_dir", "/tmp/jax_cache_kernel")
jax.config.update("jax_persistent_cache_min_entry_size_bytes", 0)
jax.config.update("jax_persistent_cache_min_compile_time_secs", 0)

import concourse.bass as bass
import concourse.mybir as mybir
import concourse.tile as tile
from concourse import bacc
from concourse.bass import ds
from concourse.bass_utils import (run_bass_kernel_spmd as _lib_run_spmd,
                                  BassKernelResults)
from concourse.masks import make_identity

BF = mybir.dt.bfloat16
F32 = mybir.dt.float32
U8 = mybir.dt.uint8
F8 = mybir.dt.float8e4
NPF8 = ml_dtypes.float8_e4m3
AF = mybir.ActivationFunctionType
OP = mybir.AluOpType
NPBF = ml_dtypes.bfloat16

B, RES, N, M, H, D, KD, NB = 4, 64, 4096, 1024, 8, 256, 32, 4
BN = B * N
NCORE = 8
NS = N // NCORE          # 512 grid tokens per core
INV_SQRT_K = float(1.0 / np.sqrt(np.float32(KD)))
ALL8 = [list(range(NCORE))]

# blob layout (per-core rows, width 1024 bf16)
BLOB_ROWS = 18           # 16 enc (4 rows x 4 batches, cols 0:512) + thr
R_ENC = 0                # rows b*4+f, cols 0:512
R_TDOWN = 16             # full (1024)
R_TUP = 17               # local shard, cols 0:512
# sheet layout (global rows, width 1024 bf16)
SHEET_ROWS = 1792        # 224 per core
SH_W1, SH_W2, SH_WR = 0, 256, 512
SH_QP, SH_KP, SH_VP = 768, 1024, 1280
SH_MISC = 1536           # cols 0:256 wde1 | 256:512 wdown | 512:768 wup
# misc2 (cols 768:1024): rows +0..4 wen; wde2 halves at cols 770,771 rows +8
# blobS cols
SC_NCD, SC_NCU, SC_BEN, SC_B1, SC_BC, SC_BD1, SCOLS = 0, 8, 16, 18, 26, 34, 36

_cache = {}
_exec_cache = {}


def run_bass_kernel_spmd(nc, in_maps, core_ids, **kw):
    """Same semantics as bass_utils.run_bass_kernel_spmd for the plain SPMD
    case, but keeps the jitted executable across calls (the library builds a
    fresh closure per call, costing ~0.1s of retrace + cache-deserialize).
    Inputs are still transferred and the NEFF executed on hardware each call.
    """
    n_cores = len(core_ids)
    if kw or list(core_ids) != list(range(n_cores)) or nc.dbg_addr is not None:
        return _lib_run_spmd(nc, in_maps, core_ids=core_ids, **kw)
    ent = _exec_cache.get(id(nc))
    if ent is None:
        from jax.sharding import Mesh, PartitionSpec
        from jax.experimental.shard_map import shard_map
        from concourse.bass2jax import (_bass_exec_p, install_neuronx_cc_hook,
                                        partition_id_tensor)
        install_neuronx_cc_hook()
        pname = (nc.partition_id_tensor.name if nc.partition_id_tensor
                 else None)
        in_names, out_names, out_avals, zero_outs = [], [], [], []
        for alloc in nc.m.functions[0].allocations:
            if not isinstance(alloc, mybir.MemoryLocationSet):
                continue
            name = alloc.memorylocations[0].name
            if alloc.kind == "ExternalInput":
                if name != pname:
                    in_names.append(name)
            elif alloc.kind == "ExternalOutput":
                out_names.append(name)
                shape = tuple(alloc.tensor_shape)
                dtype = mybir.dt.np(alloc.dtype)
                out_avals.append(jax.core.ShapedArray(shape, dtype))
                zero_outs.append(np.zeros(shape, dtype))
        n_params = len(in_names)
        all_names = in_names + out_names + ([pname] if pname else [])

        def _body(*args):
            operands = list(args)
            if pname is not None:
                operands.append(partition_id_tensor())
            outs = _bass_exec_p.bind(
                *operands, out_avals=tuple(out_avals),
                in_names=tuple(all_names), out_names=tuple(out_names),
                lowering_input_output_aliases=(), sim_require_finite=True,
                sim_require_nnan=True, nc=nc)
            return tuple(outs)

        devices = jax.devices()[:n_cores]
        mesh = Mesh(np.asarray(devices), ("core",))
        n_io = n_params + len(out_names)
        sharded = jax.jit(
            shard_map(_body, mesh=mesh,
                      in_specs=(PartitionSpec("core"),) * n_io,
                      out_specs=(PartitionSpec("core"),) * len(out_names),
                      check_rep=False),
            donate_argnums=tuple(range(n_params, n_io)), keep_unused=True)
        ent = (sharded, in_names, n_params, out_names, out_avals, zero_outs)
        _exec_cache[id(nc)] = ent
    sharded, in_names, n_params, out_names, out_avals, zero_outs = ent
    concat_in = [
        np.concatenate([np.asarray(in_maps[c][nm]) for c in range(n_cores)],
                       axis=0) for nm in in_names]
    concat_zeros = [np.zeros((n_cores * z.shape[0], *z.shape[1:]), z.dtype)
                    for z in zero_outs]
    out_arrs = sharded(*concat_in, *concat_zeros)
    results = [
        {nm: np.asarray(out_arrs[i]).reshape(n_cores, *out_avals[i].shape)[c]
         for i, nm in enumerate(out_names)}
        for c in range(n_cores)]
    return BassKernelResults(results=results, instructions_and_trace=None,
                             profile_json=None, exec_time_ns=None)


def _build():
    nc = bacc.Bacc("TRN2", target_bir_lowering=False, debug=False,
                   num_devices=NCORE)

    mcq = nc.dram_tensor("mcq", [NS, 1024], U8, kind="ExternalInput").ap()
    blob = nc.dram_tensor("blob", [BLOB_ROWS, 1024], BF,
                          kind="ExternalInput").ap()
    sheet = nc.dram_tensor("sheet", [SHEET_ROWS // NCORE, 1024], F8,
                           kind="ExternalInput").ap()
    blobS = nc.dram_tensor("blobS", [128, SCOLS], F32,
                           kind="ExternalInput").ap()
    out_shard = nc.dram_tensor("out_shard", [1, BN // NCORE], F32,
                               kind="ExternalOutput").ap()

    with tile.TileContext(nc) as tc:
        with (
            tc.tile_pool(name="dram", bufs=1, space="DRAM") as dram,
            tc.tile_pool(name="consts", bufs=1) as consts,
            tc.tile_pool(name="small", bufs=6) as small,
            tc.tile_pool(name="pp", bufs=4, space="PSUM") as pp,
            tc.tile_pool(name="pt", bufs=2, space="PSUM") as ppt,
        ):
            ident = consts.tile([128, 128], BF, name="ident", tag="ident")
            make_identity(nc, ident)
            pid = nc.sync.partition_id()

            # ---- gather the weight sheet across cores ----
            sheet_in = dram.tile([SHEET_ROWS // NCORE, 1024], F8,
                                 name="sheeti", tag="sheeti")
            nc.sync.dma_start(sheet_in[:, :], sheet[:, :])
            sheet_out = dram.tile([SHEET_ROWS, 1024], F8, name="sheeto",
                                  tag="sheeto", addr_space="Shared")
            nc.gpsimd.collective_compute(
                "AllGather", OP.bypass, replica_groups=ALL8,
                ins=[sheet_in.opt()], outs=[sheet_out.opt()])

            blobS_sb = consts.tile([128, SCOLS], F32, name="bS", tag="bS")
            nc.sync.dma_start(blobS_sb[:], blobS[:, :])

            # threshold rows broadcast to 128 partitions (ones ⊗ row matmul)
            ones_sb = consts.tile([1, 128], BF, name="ones", tag="ones")
            nc.vector.memset(ones_sb[:], 1.0)
            td_row = consts.tile([1, 1024], BF, name="tdr", tag="tdr")
            nc.sync.dma_start(td_row[:], blob[R_TDOWN:R_TDOWN + 1, :])
            thrD = consts.tile([128, 1024], BF, name="thrD", tag="thrD")
            for hf in range(2):
                pb = pp.tile([128, 512], F32, name="pp", tag="pp")
                nc.tensor.matmul(pb[:], ones_sb[:],
                                 td_row[:, hf * 512:(hf + 1) * 512])
                nc.vector.tensor_copy(thrD[:, hf * 512:(hf + 1) * 512], pb[:])
            tu_row = consts.tile([1, NS], BF, name="tur", tag="tur")
            nc.sync.dma_start(tu_row[:], blob[R_TUP:R_TUP + 1, 0:NS])
            thrU = consts.tile([128, NS], BF, name="thrU", tag="thrU")
            pb = pp.tile([128, 512], F32, name="pp", tag="pp")
            nc.tensor.matmul(pb[:], ones_sb[:], tu_row[:])
            nc.vector.tensor_copy(thrU[:], pb[:])

            def lw8(pool, p0, p1, f0, f1, name):
                # fp8 sheet load + dequant to bf16
                t8 = pool.tile([p1 - p0, f1 - f0], F8, name=name + "8",
                               tag=name + "8")
                nc.sync.dma_start(t8[:], sheet_out[p0:p1, f0:f1])
                t = pool.tile([p1 - p0, f1 - f0], BF, name=name, tag=name)
                nc.vector.tensor_copy(t[:], t8[:])
                return t

            wen_sb = lw8(consts, SH_MISC, SH_MISC + 4, 768, 1024, "wen")
            wdna, wupa = [], []
            for t in range(2):
                wdna.append(lw8(consts, SH_MISC + t * 128,
                                SH_MISC + (t + 1) * 128, 256, 512, f"wdn{t}"))
                wupa.append(lw8(consts, SH_MISC + t * 128,
                                SH_MISC + (t + 1) * 128, 512, 768,
                                f"wupt{t}"))
            ben_sb = [blobS_sb[:, SC_BEN + t:SC_BEN + t + 1] for t in range(2)]

            # local bf16 m_cross scratch (for P4's transposed reads)
            mc_loc = dram.tile([NS, 1024], BF, name="mcd", tag="mcd")
            ar_in = dram.tile([128, H * 8 * 132], F32, name="ari", tag="ari")
            ar_out = dram.tile([128, H * 8 * 132], F32, name="aro", tag="aro",
                               addr_space="Shared")
            ag3_in = dram.tile([D, M], BF, name="ag3i", tag="ag3i")
            ag3_out = dram.tile([NCORE * D, M], BF, name="ag3o", tag="ag3o",
                                addr_space="Shared")

            def psum(p, f, dt=F32):
                return pp.tile([p, f], dt, name="pp", tag="pp")

            _lwn = [0]

            def lw(pool, p0, p1, f0, f1):
                _lwn[0] += 1
                return lw8(pool, p0, p1, f0, f1, f"lw{_lwn[0]}")

            # ------- P1: down partial sums over local N-shard, all heads ----
            with tc.tile_pool(name="p1", bufs=3) as p1, \
                 tc.tile_pool(name="p1s", bufs=2) as p1s, \
                 tc.tile_pool(name="p1keep", bufs=1) as p1k:
                enc_loc = p1k.tile([4, 4 * NS], BF, name="encl", tag="encl")
                for b in range(B):
                    nc.sync.dma_start(enc_loc[:, b * NS:(b + 1) * NS],
                                      blob[R_ENC + b * 4:R_ENC + b * 4 + 4,
                                           0:NS])

                # v_all[nl]: (128, 1056) cols h*132 + b*33 + k (col 32 = ones)
                v_all = [p1k.tile([128, H * 132], BF, name=f"va{i}",
                                  tag=f"va{i}") for i in range(4)]
                for b in range(B):
                    for nl in range(4):
                        off = b * NS + nl * 128
                        enT = []
                        for t in range(2):
                            pe = psum(128, 128)
                            nc.tensor.matmul(
                                pe[:], wen_sb[:, t * 128:(t + 1) * 128],
                                enc_loc[:, off:off + 128])
                            g = p1.tile([128, 128], BF, name="enT", tag="enT")
                            nc.scalar.activation(g[:], pe[:], AF.Gelu,
                                                 bias=ben_sb[t])
                            enT.append(g)
                        pv = psum(128, 256)
                        for t in range(2):
                            nc.tensor.matmul(pv[:], enT[t][:], wdna[t][:],
                                             start=(t == 0), stop=(t == 1))
                        for h in range(H):
                            nc.vector.tensor_copy(
                                v_all[nl][:, h * 132 + b * 33:
                                          h * 132 + b * 33 + KD],
                                pv[:, h * 32:(h + 1) * 32])
                        if b == 0:
                            for h in range(H):
                                for bb in range(B):
                                    nc.vector.memset(
                                        v_all[nl][:, h * 132 + bb * 33 + 32:
                                                  h * 132 + bb * 33 + 33],
                                        1.0)

                # local mc tiles: u8 -> bf16 (+ write-through for P4), masks
                mct, msk = [], []
                for nl in range(4):
                    mq = p1.tile([128, M], U8, name="mq", tag="mq")
                    nc.sync.dma_start(mq[:], mcq[nl * 128:(nl + 1) * 128, :])
                    t = p1k.tile([128, M], BF, name=f"mct{nl}", tag=f"mct{nl}")
                    nc.vector.tensor_copy(t[:], mq[:])
                    nc.sync.dma_start(mc_loc[nl * 128:(nl + 1) * 128, :], t[:])
                    m = p1k.tile([128, M], BF, name=f"msk{nl}", tag=f"msk{nl}")
                    nc.vector.tensor_tensor(m[:], t[:], thrD[:], OP.is_le)
                    mct.append(t)
                    msk.append(m)

                # per head: a = exp(-c_h/255 q) * mask; px partials -> pxs
                pxs = p1k.tile([128, H * 8 * 132], F32, name="pxs", tag="pxs")
                for h in range(H):
                    ah = [p1s.tile([128, M], BF, name=f"ah{nl}", tag=f"ah{nl}")
                          for nl in range(4)]
                    for nl in range(4):
                        nc.scalar.activation(
                            ah[nl][:], mct[nl][:], AF.Exp,
                            scale=blobS_sb[:, SC_NCD + h:SC_NCD + h + 1])
                        nc.vector.tensor_tensor(ah[nl][:], ah[nl][:],
                                                msk[nl][:], OP.mult)
                    for mi in range(8):
                        pa = psum(128, 132)
                        for nl in range(4):
                            nc.tensor.matmul(
                                pa[:], ah[nl][:, mi * 128:(mi + 1) * 128],
                                v_all[nl][:, h * 132:(h + 1) * 132],
                                start=(nl == 0), stop=(nl == 3))
                        nc.vector.tensor_copy(
                            pxs[:, (h * 8 + mi) * 132:(h * 8 + mi + 1) * 132],
                            pa[:])
                nc.sync.dma_start(ar_in[:], pxs[:])
                nc.gpsimd.collective_compute(
                    "AllReduce", OP.add, replica_groups=ALL8,
                    ins=[ar_in.opt()], outs=[ar_out.opt()])

            # ---------------- P2: blocks ----------------
            # every core computes ALL 8 heads for its batch (pid//2)
            b0c = (pid // 2) * 33
            with tc.tile_pool(name="p2", bufs=2) as p2, \
                 tc.tile_pool(name="p2s", bufs=2) as p2s, \
                 tc.tile_pool(name="p2e", bufs=1) as p2e:
                # finalize x^T for my batch from the reduced sums
                xT = [p2e.tile([128, M], BF, name=f"xT{t}", tag=f"xT{t}")
                      for t in range(2)]
                for h in range(H):
                    for mi in range(8):
                        nb = small.tile([128, 33], F32, name="nb", tag="nb")
                        nc.sync.dma_start(
                            nb[:], ar_out[0:128,
                                          ds((h * 8 + mi) * 132 + b0c, 33)])
                        rc = small.tile([128, 1], F32, name="rc", tag="rc")
                        nc.vector.reciprocal(rc[:], nb[:, 32:33])
                        gx = small.tile([128, KD], BF, name="gx", tag="gx")
                        nc.scalar.activation(gx[:], nb[:, 0:KD], AF.Gelu,
                                             scale=rc[:])
                        ptr = ppt.tile([KD, 128], BF, name="tp", tag="tp")
                        nc.tensor.transpose(ptr[:], gx[:], ident[:])
                        nc.vector.tensor_copy(
                            xT[h // 4][(h % 4) * 32:(h % 4) * 32 + 32,
                                       mi * 128:(mi + 1) * 128], ptr[:])

                for blk in range(NB):
                    qp_sb, kp_sb, vp_sb = [], [], []
                    for t in range(2):
                        for dst, base in ((qp_sb, SH_QP), (kp_sb, SH_KP),
                                          (vp_sb, SH_VP)):
                            dst.append(lw(p2, base + t * 128,
                                          base + (t + 1) * 128,
                                          blk * 256, (blk + 1) * 256))

                    qt = [p2e.tile([128, M], BF, name=f"qt{g}", tag=f"qt{g}")
                          for g in range(2)]
                    kt = [p2e.tile([128, M], BF, name=f"kt{g}", tag=f"kt{g}")
                          for g in range(2)]
                    for dst2, wsb in ((qt, qp_sb), (kt, kp_sb)):
                        for g in range(2):
                            for mh in range(2):
                                pq = psum(128, 512)
                                for t in range(2):
                                    nc.tensor.matmul(
                                        pq[:],
                                        wsb[t][:, g * 128:(g + 1) * 128],
                                        xT[t][:, mh * 512:(mh + 1) * 512],
                                        start=(t == 0), stop=(t == 1))
                                nc.vector.tensor_copy(
                                    dst2[g][:, mh * 512:(mh + 1) * 512],
                                    pq[:])
                    qh = [p2e.tile([KD, M], BF, name=f"qh{h}", tag=f"qh{h}")
                          for h in range(H)]
                    kh = [p2e.tile([KD, M], BF, name=f"kh{h}", tag=f"kh{h}")
                          for h in range(H)]
                    for h in range(H):
                        nc.vector.tensor_copy(
                            qh[h][:], qt[h // 4][(h % 4) * 32:
                                                 (h % 4) * 32 + 32, :])
                        nc.vector.tensor_copy(
                            kh[h][:], kt[h // 4][(h % 4) * 32:
                                                 (h % 4) * 32 + 32, :])

                    vh = [p2e.tile([128, H * 33], BF, name=f"vh{ni}",
                                   tag=f"vh{ni}") for ni in range(8)]
                    for ni in range(8):
                        pvv = psum(128, 256)
                        for t in range(2):
                            nc.tensor.matmul(
                                pvv[:],
                                xT[t][:, ni * 128:(ni + 1) * 128],
                                vp_sb[t][:], start=(t == 0), stop=(t == 1))
                        for h in range(H):
                            nc.vector.tensor_copy(
                                vh[ni][:, h * 33:h * 33 + KD],
                                pvv[:, h * 32:h * 32 + 32])
                            nc.vector.memset(
                                vh[ni][:, h * 33 + 32:h * 33 + 33], 1.0)

                    paT = [p2e.tile([128, M], BF, name=f"paT{t}",
                                    tag=f"paT{t}") for t in range(2)]
                    for h in range(H):
                        es = [p2s.tile([128, M], BF, name=f"es{ni}",
                                       tag=f"es{ni}") for ni in range(8)]
                        for ni in range(8):
                            for mh in range(2):
                                psc = psum(128, 512)
                                nc.tensor.matmul(
                                    psc[:],
                                    kh[h][:, ni * 128:(ni + 1) * 128],
                                    qh[h][:, mh * 512:(mh + 1) * 512])
                                nc.scalar.activation(
                                    es[ni][:, mh * 512:(mh + 1) * 512],
                                    psc[:], AF.Exp, scale=INV_SQRT_K)
                        for mi in range(8):
                            pa = psum(128, 33)
                            for ni in range(8):
                                nc.tensor.matmul(
                                    pa[:],
                                    es[ni][:, mi * 128:(mi + 1) * 128],
                                    vh[ni][:, h * 33:h * 33 + 33],
                                    start=(ni == 0), stop=(ni == 7))
                            rc = small.tile([128, 1], F32, name="rc", tag="rc")
                            nc.vector.reciprocal(rc[:], pa[:, 32:33])
                            gx = small.tile([128, KD], BF, name="gx", tag="gx")
                            nc.scalar.activation(gx[:], pa[:, 0:KD], AF.Gelu,
                                                 scale=rc[:])
                            ptr = ppt.tile([KD, 128], BF, name="tp", tag="tp")
                            nc.tensor.transpose(ptr[:], gx[:], ident[:])
                            nc.vector.tensor_copy(
                                paT[h // 4][(h % 4) * 32:(h % 4) * 32 + 32,
                                            mi * 128:(mi + 1) * 128], ptr[:])

                    w1_sb = [[lw(p2, SH_W1 + i * 128, SH_W1 + (i + 1) * 128,
                                 blk * 256 + o * 128, blk * 256 + (o + 1) * 128)
                              for o in range(2)] for i in range(2)]
                    b1_sb = [blobS_sb[:, SC_B1 + blk * 2 + t:
                                      SC_B1 + blk * 2 + t + 1]
                             for t in range(2)]
                    h1 = [p2e.tile([128, M], BF, name=f"h1{t}", tag=f"h1{t}")
                          for t in range(2)]
                    for o in range(2):
                        for mh in range(2):
                            ph = psum(128, 512)
                            for i in range(2):
                                nc.tensor.matmul(
                                    ph[:], w1_sb[i][o][:],
                                    paT[i][:, mh * 512:(mh + 1) * 512],
                                    start=(i == 0), stop=(i == 1))
                            nc.scalar.activation(
                                h1[o][:, mh * 512:(mh + 1) * 512], ph[:],
                                AF.Gelu, bias=b1_sb[o])

                    w2_sb = [[lw(p2, SH_W2 + i * 128, SH_W2 + (i + 1) * 128,
                                 blk * 256 + o * 128, blk * 256 + (o + 1) * 128)
                              for o in range(2)] for i in range(2)]
                    wr_sb = [[lw(p2, SH_WR + i * 128, SH_WR + (i + 1) * 128,
                                 blk * 256 + o * 128, blk * 256 + (o + 1) * 128)
                              for o in range(2)] for i in range(2)]
                    bc_sb = [blobS_sb[:, SC_BC + blk * 2 + t:
                                      SC_BC + blk * 2 + t + 1]
                             for t in range(2)]
                    xn = [p2e.tile([128, M], BF, name=f"xn{t}", tag=f"xn{t}")
                          for t in range(2)]
                    for o in range(2):
                        for mh in range(2):
                            po = psum(128, 512)
                            nc.tensor.matmul(
                                po[:], w2_sb[0][o][:],
                                h1[0][:, mh * 512:(mh + 1) * 512],
                                start=True, stop=False)
                            nc.tensor.matmul(
                                po[:], w2_sb[1][o][:],
                                h1[1][:, mh * 512:(mh + 1) * 512],
                                start=False, stop=False)
                            nc.tensor.matmul(
                                po[:], wr_sb[0][o][:],
                                xT[0][:, mh * 512:(mh + 1) * 512],
                                start=False, stop=False)
                            nc.tensor.matmul(
                                po[:], wr_sb[1][o][:],
                                xT[1][:, mh * 512:(mh + 1) * 512],
                                start=False, stop=True)
                            nc.scalar.activation(
                                xn[o][:, mh * 512:(mh + 1) * 512], po[:],
                                AF.Gelu, bias=bc_sb[o])
                    xT = xn

                for t in range(2):
                    nc.sync.dma_start(ag3_in[t * 128:(t + 1) * 128, :],
                                      xT[t][:])
                nc.gpsimd.collective_compute(
                    "AllGather", OP.bypass, replica_groups=ALL8,
                    ins=[ag3_in.opt()], outs=[ag3_out.opt()])

            # ------- P4+P5: up on local token shard (all heads) + decoder ---
            TS = BN // NCORE  # 2048 = 4 batches x 512 local tokens
            with tc.tile_pool(name="p4", bufs=3) as p4, \
                 tc.tile_pool(name="p4s", bufs=2) as p4s, \
                 tc.tile_pool(name="p4keep", bufs=1) as p4k:
                # vu_all[mi]: (128, 1056) cols h*132 + b*33 + k
                vu = [p4k.tile([128, H * 132], BF, name=f"vu{i}",
                               tag=f"vu{i}") for i in range(8)]
                for b in range(B):
                    xb = [p4.tile([128, M], BF, name=f"xb{t}", tag=f"xb{t}")
                          for t in range(2)]
                    for t in range(2):
                        nc.sync.dma_start(
                            xb[t][:],
                            ag3_out[2 * b * D + t * 128:
                                    2 * b * D + (t + 1) * 128, :])
                    for mi in range(8):
                        pv = psum(128, 256)
                        for t in range(2):
                            nc.tensor.matmul(
                                pv[:], xb[t][:, mi * 128:(mi + 1) * 128],
                                wupa[t][:], start=(t == 0), stop=(t == 1))
                        for h in range(H):
                            nc.vector.tensor_copy(
                                vu[mi][:, h * 132 + b * 33:
                                       h * 132 + b * 33 + KD],
                                pv[:, h * 32:(h + 1) * 32])
                        if b == 0:
                            for h in range(H):
                                for bb in range(B):
                                    nc.vector.memset(
                                        vu[mi][:, h * 132 + bb * 33 + 32:
                                               h * 132 + bb * 33 + 33], 1.0)

                # local mc^T tiles (128 M-part, 512 local N) + masks
                mctT, mskU = [], []
                for ki in range(8):
                    t = p4k.tile([128, NS], BF, name=f"mT{ki}", tag=f"mT{ki}")
                    nc.sync.dma_start(t[:],
                                      mc_loc[:, ki * 128:(ki + 1) * 128],
                                      transpose=True)
                    m = p4k.tile([128, NS], BF, name=f"mU{ki}", tag=f"mU{ki}")
                    nc.vector.tensor_tensor(m[:], t[:], thrU[:], OP.is_le)
                    mctT.append(t)
                    mskU.append(m)

                deT = [p4k.tile([128, TS], BF, name=f"deT{t}", tag=f"deT{t}")
                       for t in range(2)]
                for h in range(H):
                    eh = [p4s.tile([128, NS], BF, name=f"eh{ki}",
                                   tag=f"eh{ki}") for ki in range(8)]
                    for ki in range(8):
                        nc.scalar.activation(
                            eh[ki][:], mctT[ki][:], AF.Exp,
                            scale=blobS_sb[:, SC_NCU + h:SC_NCU + h + 1])
                        nc.vector.tensor_tensor(eh[ki][:], eh[ki][:],
                                                mskU[ki][:], OP.mult)
                    for qi in range(4):
                        pd = psum(128, 132)
                        for ki in range(8):
                            nc.tensor.matmul(
                                pd[:], eh[ki][:, qi * 128:(qi + 1) * 128],
                                vu[ki][:, h * 132:(h + 1) * 132],
                                start=(ki == 0), stop=(ki == 7))
                        for b in range(B):
                            rc = small.tile([128, 1], F32, name="rc", tag="rc")
                            nc.vector.reciprocal(
                                rc[:], pd[:, b * 33 + 32:b * 33 + 33])
                            gx = small.tile([128, KD], BF, name="gx", tag="gx")
                            nc.scalar.activation(
                                gx[:], pd[:, b * 33:b * 33 + KD],
                                AF.Gelu, scale=rc[:])
                            ptr = ppt.tile([KD, 128], BF, name="tp", tag="tp")
                            nc.tensor.transpose(ptr[:], gx[:], ident[:])
                            nc.vector.tensor_copy(
                                deT[h // 4][(h % 4) * 32:(h % 4) * 32 + 32,
                                            b * NS + qi * 128:
                                            b * NS + (qi + 1) * 128], ptr[:])

                # decoder directly on the local de^T tiles
                wd1 = [[lw(p4, SH_MISC + i * 128, SH_MISC + (i + 1) * 128,
                           o * 128, (o + 1) * 128) for o in range(2)]
                       for i in range(2)]
                bd1 = [blobS_sb[:, SC_BD1 + t:SC_BD1 + t + 1]
                       for t in range(2)]
                wd2 = [lw(p4, SH_MISC + 8, SH_MISC + 8 + 128,
                          770 + t, 771 + t) for t in range(2)]
                g = [p4k.tile([128, TS], BF, name=f"g{t}", tag=f"g{t}")
                     for t in range(2)]
                for o in range(2):
                    for th in range(4):
                        pg = psum(128, 512)
                        for i in range(2):
                            nc.tensor.matmul(
                                pg[:], wd1[i][o][:],
                                deT[i][:, th * 512:(th + 1) * 512],
                                start=(i == 0), stop=(i == 1))
                        nc.scalar.activation(
                            g[o][:, th * 512:(th + 1) * 512], pg[:],
                            AF.Gelu, bias=bd1[o])
                osb = p4k.tile([1, TS], F32, name="osb", tag="osb")
                for th in range(4):
                    p2o = psum(1, 512)
                    for i in range(2):
                        nc.tensor.matmul(
                            p2o[:], wd2[i][:],
                            g[i][:, th * 512:(th + 1) * 512],
                            start=(i == 0), stop=(i == 1))
                    nc.vector.tensor_copy(
                        osb[:, th * 512:(th + 1) * 512], p2o[:])
                nc.sync.dma_start(out_shard[:, :], osb[:])

    nc.compile()
    return nc


def _prep_inputs(inputs, m_cross, W_en, b_en, r_down, w_down, q_pa, k_pa,
                 v_pa, W1_mlp, b1_mlp, W2_mlp, b2_mlp, W_res, b_res, r_up,
                 w_up, W_de1, b_de1, W_de2, b_de2, y_mean, y_std):
    f32 = np.float32
    mc = np.asarray(m_cross, f32)
    # uint8 quantization: q = round(mc*255); same absolute precision as bf16
    # on [0,1) and exact integer threshold comparisons on device
    mcq8 = np.clip(np.round(mc * 255.0), 0, 255).astype(np.uint8)
    mcqf = mcq8.astype(f32)

    # quantization-robust percentile thresholds: max passing q value so the
    # device-side (q <= thr) comparison reproduces the f32 mask
    kd_ = int(0.30 * (N - 1))          # 1228
    vkd = np.partition(mc, kd_, axis=0)[kd_, :]               # (M,)
    t_down_dev = np.where(mc <= vkd[None, :], mcqf, -np.inf).max(axis=0)
    ku_ = int(0.30 * (M - 1))          # 306
    vku = np.partition(mc, ku_, axis=1)[:, ku_]               # (N,)
    t_up_dev = np.where(mc <= vku[:, None], mcqf, -np.inf).max(axis=1)

    # encoder input rows (4 feats x BN), sliced per core below
    gx = np.linspace(0.0, 1.0, RES + 1, dtype=f32)[:-1]
    gxx = np.broadcast_to(gx[:, None], (RES, RES))
    gyy = np.broadcast_to(gx[None, :], (RES, RES))
    encf = np.zeros((4, B, N), f32)
    encf[0] = gxx.reshape(-1)[None, :]
    encf[1] = gyy.reshape(-1)[None, :]
    encf[2] = np.asarray(inputs, f32).reshape(B, N)

    c_down = np.tan(0.25 * np.pi * (1.0 + np.sin(np.asarray(r_down, f32)
                                                 .reshape(H)))).astype(f32)
    c_up = np.tan(0.25 * np.pi * (1.0 + np.sin(np.asarray(r_up, f32)
                                               .reshape(H)))).astype(f32)

    # ---- weight sheet (1792, 1024) ----
    sheet = np.zeros((SHEET_ROWS, 1024), NPF8)

    def cat_blocks(w):  # (NB, D, D) -> (D, NB*D)
        return np.asarray(w, f32).transpose(1, 0, 2).reshape(D, NB * D)

    def cat_heads(w):   # (NB, H, D, K) -> (D, NB*H*K)
        return np.asarray(w, f32).transpose(2, 0, 1, 3).reshape(D, NB * H * KD)

    sheet[SH_W1:SH_W1 + D] = cat_blocks(W1_mlp).astype(NPF8)
    sheet[SH_W2:SH_W2 + D] = cat_blocks(W2_mlp).astype(NPF8)
    sheet[SH_WR:SH_WR + D] = cat_blocks(W_res).astype(NPF8)
    sheet[SH_QP:SH_QP + D] = cat_heads(q_pa).astype(NPF8)
    sheet[SH_KP:SH_KP + D] = cat_heads(k_pa).astype(NPF8)
    sheet[SH_VP:SH_VP + D] = cat_heads(v_pa).astype(NPF8)
    sheet[SH_MISC:SH_MISC + D, 0:256] = np.asarray(W_de1, f32).astype(NPF8)
    sheet[SH_MISC:SH_MISC + D, 256:512] = (
        np.asarray(w_down, f32).transpose(1, 0, 2).reshape(D, H * KD)
        .astype(NPF8))
    sheet[SH_MISC:SH_MISC + D, 512:768] = (
        np.asarray(w_up, f32).transpose(1, 0, 2).reshape(D, H * KD)
        .astype(NPF8))
    wen4 = np.zeros((4, D), f32)
    wen4[:3, :] = np.asarray(W_en, f32)
    sheet[SH_MISC:SH_MISC + 4, 768:1024] = wen4.astype(NPF8)
    ystd = float(np.asarray(y_std, f32))
    ymean = float(np.asarray(y_mean, f32))
    wde2f = (np.asarray(W_de2, f32).reshape(D) * ystd).astype(NPF8)
    sheet[SH_MISC + 8:SH_MISC + 8 + 128, 770] = wde2f[0:128]
    sheet[SH_MISC + 8:SH_MISC + 8 + 128, 771] = wde2f[128:256]
    bde2f = float(np.asarray(b_de2, f32).reshape(-1)[0] * ystd + ymean)

    # ---- f32 constants (128, 36) ----  (exp scales absorb the /255)
    bS = np.zeros((128, SCOLS), f32)
    bS[:, SC_NCD:SC_NCD + 8] = -c_down[None, :] / 255.0
    bS[:, SC_NCU:SC_NCU + 8] = -c_up[None, :] / 255.0
    ben = np.asarray(b_en, f32).reshape(D)
    bS[:, SC_BEN] = ben[0:128]
    bS[:, SC_BEN + 1] = ben[128:256]
    b1f = np.asarray(b1_mlp, f32).reshape(NB, D)
    bcf = (np.asarray(b2_mlp, f32) + np.asarray(b_res, f32)).reshape(NB, D)
    for blk in range(NB):
        for t in range(2):
            bS[:, SC_B1 + blk * 2 + t] = b1f[blk, t * 128:(t + 1) * 128]
            bS[:, SC_BC + blk * 2 + t] = bcf[blk, t * 128:(t + 1) * 128]
    bd1 = np.asarray(b_de1, f32).reshape(D)
    bS[:, SC_BD1] = bd1[0:128]
    bS[:, SC_BD1 + 1] = bd1[128:256]

    in_maps = []
    for c in range(NCORE):
        blob = np.zeros((BLOB_ROWS, 1024), NPBF)
        for b in range(B):
            blob[R_ENC + b * 4:R_ENC + b * 4 + 4, 0:NS] = (
                encf[:, b, c * NS:(c + 1) * NS].astype(NPBF))
        blob[R_TDOWN] = t_down_dev.astype(NPBF)
        blob[R_TUP, 0:NS] = t_up_dev[c * NS:(c + 1) * NS].astype(NPBF)
        in_maps.append({
            "mcq": mcq8[c * NS:(c + 1) * NS],
            "blob": blob,
            "sheet": sheet[c * (SHEET_ROWS // NCORE):
                           (c + 1) * (SHEET_ROWS // NCORE)],
            "blobS": bS,
        })
    return in_maps, bde2f


def kernel(**inputs):
    if "nc" not in _cache:
        _cache["nc"] = _build()
    nc = _cache["nc"]
    in_maps, bde2f = _prep_inputs(**inputs)
    res = run_bass_kernel_spmd(nc, in_maps, core_ids=list(range(NCORE)))
    out = np.empty((B, N), np.float32)
    for c in range(NCORE):
        sh = res.results[c]["out_shard"].reshape(B, NS) + np.float32(bde2f)
        out[:, c * NS:(c + 1) * NS] = sh
    return out.reshape(B, RES, RES, 1).astype(np.float32)
